# revision 25
# baseline (speedup 1.0000x reference)
"""Conv2d(128->256, 3x3, pad 1) with LoRA (rank 8) — Trainium2 Bass kernel.

Strategy:
  - Data-parallel over batch: 16 images -> 2 per core x 8 cores. Conv weights
    and LoRA A/B replicated.
  - LoRA folds into the conv weight on device (conv is linear in weights):
        W_eff = W + (alpha/rank) * (B @ A).reshape(C_OUT, C_IN, 3, 3)
    via 9 tiny PE matmuls (K=8) + fused DVE scalar_tensor_tensor adds.
  - The 3x3 conv = 9 shifted matmuls accumulating in PSUM. Seven taps run
    bf16 (1 col/cycle); taps (0,2) are PACKED into ONE fp8e4 DoubleRow
    matmul (the PE virtualizes to 128x256: two fp8 weights/cell, two
    multiplies/cycle), so each output tile takes 8 matmul slots instead
    of 9 — an 11% cut of the matmul stream. The two packed taps share the
    x row window (both kh=0, kw=0 vs kw=2), expressed as a 4D ifmap AP
    with a 2-byte slot stride; weights are e4m3 quantized from weff on
    ACT (256-col slot stride). Measured hybrid rel err 1.67e-2 < 2e-2.
  - All tensor I/O is bf16 (fp8 for the packed-tap x copy); output is
    written bf16 and upcast on host.
  - Head: three f32 warm-up matmuls (on a 128-col memset tile read through
    a stride-0 broadcast AP) release the PE clock gate (HAM) before the
    conv; ab lands first -> LoRA matmuls; wt arrives in five fold-order
    pieces over all three DMA queues; the conv is emitted in k-minor
    2-row-group waves chasing the weff folds.
  - Tail: the final row group drains as 8+4+2+2-row units, k-major, so
    only 32KB lands after the last conv matmul; three dep-free f32 filler
    matmuls then keep the HAM activity monitor at full clock while the
    framework's end-of-kernel semaphore-reset parade (~60 sems/engine,
    which the NTFF exec-time window includes) runs — at half clock it
    costs ~9us, at full ~4.5us.
"""

import numpy as np
import ml_dtypes

import concourse.bass as bass
import concourse.tile as tile
from concourse.tile import add_dep_helper
from concourse import bacc, mybir
from concourse.ap import AP
from concourse.bass_utils import run_bass_kernel_spmd

N_CORES = 8
B, C_IN, H, W_DIM = 16, 128, 64, 64
C_OUT = 256
RANK = 8
SCALING = 2.0  # alpha/rank = 16/8
HP, WP = H + 2, W_DIM + 2  # zero-padded image dims
B_LOC = B // N_CORES  # images per core
NPIX = H * W_DIM  # 4096
ROWS_PER_TILE = 8  # output rows per matmul group -> N = 8*64 = 512
N_RG = H // ROWS_PER_TILE  # 8 row groups

F32 = mybir.dt.float32
BF16 = mybir.dt.bfloat16
F8E4 = mybir.dt.float8e4
IDENT = mybir.ActivationFunctionType.Identity
DR = mybir.MatmulPerfMode.DoubleRow
BF16_NP = ml_dtypes.bfloat16
E4_NP = ml_dtypes.float8_e4m3

# taps 0 and 2 (kh=0, kw=0/2) run packed fp8; the rest bf16.
BF_TAPS = [1, 3, 4, 5, 6, 7, 8]


def _build_nc():
    nc = bacc.Bacc(
        "TRN2",
        target_bir_lowering=False,
        debug=False,
        num_devices=N_CORES,
    )

    xp = nc.dram_tensor("xp", [B_LOC, C_IN, HP * WP], BF16, kind="ExternalInput").ap()
    wt = nc.dram_tensor("wt", [C_IN, 9 * C_OUT], BF16, kind="ExternalInput").ap()
    # at, bt and 256 zero columns bundled: [8, 9*128 | 256 | 256] -> one DMA.
    ab = nc.dram_tensor(
        "ab", [RANK, 9 * C_IN + 2 * C_OUT], BF16, kind="ExternalInput"
    ).ap()
    bv = nc.dram_tensor("bv", [128, 2], F32, kind="ExternalInput").ap()
    out = nc.dram_tensor("out", [B_LOC, C_OUT, NPIX], BF16, kind="ExternalOutput").ap()

    with tile.TileContext(nc) as tc:
        with (
            tc.tile_pool(name="persist", bufs=1) as persist,
            tc.tile_pool(name="outp", bufs=6) as outp,
            tc.tile_pool(name="psum", bufs=8, space="PSUM") as psum,
        ):
            # --- persistent SBUF tiles ------------------------------------
            x_sb = [
                persist.tile([C_IN, HP * WP], BF16, name=f"x_sb{i}")
                for i in range(B_LOC)
            ]
            x8_sb = [
                persist.tile([C_IN, HP * WP], F8E4, name=f"x8_sb{i}")
                for i in range(B_LOC)
            ]
            wt_sb = persist.tile([C_IN, 9 * C_OUT], BF16, name="wt_sb")
            weff = persist.tile([C_IN, 9 * C_OUT], BF16, name="weff")
            # packed fp8 weights: cols 0:256 = tap0, 256:512 = tap2
            weff8 = persist.tile([C_IN, 2 * C_OUT], F8E4, name="weff8")
            ab_sb = persist.tile([RANK, 9 * C_IN + 2 * C_OUT], BF16, name="ab_sb")
            b_sb = persist.tile([128, 2], F32, name="b_sb")
            warm_sb = persist.tile([128, 128], F32, name="warm_sb")
            # stride-0 broadcast read: 4x repeat of the 128 cols -> N=512
            warm_bcast = AP(
                warm_sb[:].tensor,
                warm_sb[:].offset,
                [warm_sb[:].ap[0], [0, 4], [1, 128]],
            )

            # --- input DMAs ------------------------------------------------
            # Queue FIFO order = priority order; each DMA_DIRECT2D costs
            # ~0.65us of issue time on its queue engine and completion sems
            # lag the data by ~1.5-2us. Critical path to the first conv
            # matmul: ab -> LoRA MMs -> (with wt p0) weff fold 0.
            qs = [nc.sync, nc.scalar]
            #   sync:   wt p0, ab, x8 img0 rows 0-15, rows 16-31
            #   scalar: x0 wave A rows, wt p2, x0 rest, x1, x8 img1
            #   gpsimd (SWDGE): wt p1, p3, p4, bias, x8 img0 rows 32-65
            # The fp8 x copies are DERIVED ON DEVICE from the bf16 x tiles
            # (ACT converting copies, interleaved with the drains) — no x8
            # DMA traffic at all.
            #   sync:   wt p0, ab, wt p4
            #   scalar: x0 rows 0-17/18-33, wt p2, x0 rows 50-65, x1 (2 pc)
            #   gpsimd: wt p1, wt p3, bias, x0 rows 34-49
            nc.sync.dma_start(wt_sb[:, 256:512], wt[:, 256:512])
            nc.sync.dma_start(ab_sb[:], ab)
            nc.sync.dma_start(wt_sb[:, 0:256], wt[:, 0:256])
            nc.sync.dma_start(wt_sb[:, 2048:], wt[:, 2048:])
            xa1 = 18 * WP  # rows 0..17: conv rg0-1 (+ rg2's upper rows)
            xa = 34 * WP  # rows 0..33 cover conv wave A+B (rg0-3)
            xc = 50 * WP  # rows 34..49 (wave C) ride gpsimd; 50..65 scalar
            nc.scalar.dma_start(x_sb[0][:, :xa1], xp[0, :, :xa1])
            nc.scalar.dma_start(x_sb[0][:, xa1:xa], xp[0, :, xa1:xa])
            nc.scalar.dma_start(wt_sb[:, 1024:1536], wt[:, 1024:1536])
            nc.scalar.dma_start(x_sb[0][:, xc:], xp[0, :, xc:])
            nc.scalar.dma_start(x_sb[1][:, : xa], xp[1, :, :xa])
            nc.scalar.dma_start(x_sb[1][:, xa:], xp[1, :, xa:])
            # gpsimd queue is ready ~1.3us before the DVE's; the tiny warm
            # memset (128 cols) gates the f32 warm-up matmuls.
            nc.gpsimd.memset(warm_sb[:], 0.0)
            nc.gpsimd.dma_start(wt_sb[:, 512:1024], wt[:, 512:1024])
            nc.gpsimd.dma_start(wt_sb[:, 1536:2048], wt[:, 1536:2048])
            nc.gpsimd.dma_start(b_sb[:], bv)
            nc.gpsimd.dma_start(x_sb[0][:, xa:xc], xp[0, :, xa:xc])

            # --- PE warm-up ------------------------------------------------
            # The HAM clock gate holds the PE at 1.2 GHz until ~3.4us of
            # sustained busy. Three f32 N=512 matmuls (4 cycles/row, no DMA
            # deps) reliably release it.
            lps = [
                psum.tile([128, 512], F32, tag="lps", bufs=3, name=f"lps{j}")
                for j in range(5)
            ]
            # Three f32 N=512 warms (each lowers to TWO HW passes): the PE
            # must stay busy with NO gap until the ab DMA lands and LoRA
            # takes over, or the HAM restarts its 3.4us sustained-busy
            # requirement and the early conv runs at 1.2 GHz.
            for _ in range(3):
                nc.tensor.matmul(
                    lps[0][:], warm_sb[:], warm_bcast, start=True, stop=True
                )

            # --- fold LoRA into the conv weight ----------------------------
            # lps[j][:, (k%2)*256:...] = (A_k)^T @ B^T  for k = 2j, 2j+1
            # weff[:, k*256+co] = wt[:, k*256+co] + 2 * lps[...]
            # Tile dependency tracking is backward-looking: k0..k5 and folds
            # A..C are emitted here; k6..k8 AND folds D/E are emitted inside
            # conv wave A after its first row.
            def lora_mm(k):
                nc.tensor.matmul(
                    lps[k // 2][:, (k % 2) * 256 : (k % 2) * 256 + 256],
                    ab_sb[:, k * 128 : (k + 1) * 128],
                    ab_sb[:, 9 * C_IN : 9 * C_IN + 256],
                    start=True,
                    stop=True,
                )

            # k1 first: tap1 is the first conv tap, and its weff fold is
            # gated by this matmul + the (64KB, first-on-sync) wt piece.
            # k6..k8 wait on fold A freeing an lps bank (bufs=3); emitting
            # everything pre-conv lets folds D/E start the moment wt p3/p4
            # land instead of mid-wave-A.
            for k in [1, 0, 2, 3, 4, 5, 6, 7, 8]:
                lora_mm(k)

            def chain(inst, prev, why):
                if prev is not None:
                    add_dep_helper(inst.ins, prev.ins, sync=False, reason=why)
                return inst

            def fold_cols(j, c0, w, link):
                return chain(
                    nc.vector.scalar_tensor_tensor(
                        weff[:, j * 512 + c0 : j * 512 + c0 + w],
                        lps[j][:, c0 : c0 + w],
                        SCALING,
                        wt_sb[:, j * 512 + c0 : j * 512 + c0 + w],
                        op0=mybir.AluOpType.mult,
                        op1=mybir.AluOpType.add,
                    ),
                    link,
                    "weff fold k order",
                )

            # tap1's half of fold A runs first — it gates the first conv
            # matmul; tap0's half only gates the weff8 quantize.
            link = fold_cols(0, 256, 256, None)
            link = fold_cols(0, 0, 256, link)
            for j in range(1, 5):
                link = fold_cols(j, 0, 512 if j < 4 else 256, link)
            # quantize packed-tap weights (tap0 <- fold0, tap2 <- fold1) on
            # ACT, which is idle until the first drain.
            nc.scalar.copy(weff8[:, 0:256], weff[:, 0:256])
            nc.scalar.copy(weff8[:, 256:512], weff[:, 512:768])
            # fp8 x img0, derived on ACT from the bf16 tiles as their DMA
            # pieces land. Rows 16-31 first: wave 1's DoubleRow needs them
            # at ~15us; rows 0-15 are only consumed from cb1's first wave
            # (the very first wave runs all-bf16 instead).
            nc.scalar.copy(x8_sb[0][:, 16 * WP : 32 * WP], x_sb[0][:, 16 * WP : 32 * WP])
            nc.scalar.copy(x8_sb[0][:, : 16 * WP], x_sb[0][:, : 16 * WP])

            w8r = weff8[:].rearrange("p (s m) -> p s m", s=2)

            # --- the conv: 8 matmuls per output tile (7 bf16 + 1 DoubleRow)
            first_wave = True
            for img in range(B_LOC):
                x_r = x_sb[img][:].rearrange("p (h w) -> p h w", w=WP)
                x8_r = x8_sb[img][:].rearrange("p (h w) -> p h w", w=WP)
                for cb in range(2):
                    for wv, rgs in enumerate(([0, 1], [2, 3], [4, 5], [6, 7])):
                        last_wave = img == B_LOC - 1 and cb == 1 and wv == 3
                        # units: (rg, row offset within rg, n rows).
                        if not last_wave:
                            units = [(rg, 0, 8) for rg in rgs]
                        else:
                            units = [(6, 0, 8), (7, 0, 4), (7, 4, 2), (7, 6, 2)]
                        ps = {
                            u: psum.tile(
                                [128, u[2] * 64], F32, tag="ps", bufs=5,
                                name=f"ps{img}_{cb}_{u[0]}_{u[1]}",
                            )
                            for u in units
                        }
                        # normal waves: k-minor. The bf16<->DoubleRow mode
                        # switch costs ~190ns (DR LDWEIGHTS can't overlap a
                        # bf16 matmul), so DR rows PAIR across wave
                        # boundaries: waves 0/2 put the DR row last, waves
                        # 1/3 first -> the 4 DR matmuls run back-to-back and
                        # the penalty halves. Wave 0 DR-last also gives the
                        # x8 completion sems time in the chase window.
                        # Last wave: k-major per unit, DoubleRow first so
                        # the final unit's drain chain is short.
                        if img == 0 and cb == 0 and wv == 0:
                            # all-bf16 first wave: its x8/weff8 inputs would
                            # gate the PE >2.2us and trip the HAM throttle
                            klist = BF_TAPS + [0, 2]
                            kloop = [(k, u) for k in klist for u in units]
                        elif not last_wave:
                            if wv % 2 == 0:
                                klist = BF_TAPS + ["DR"]
                            else:
                                klist = ["DR"] + BF_TAPS
                            kloop = [(k, u) for k in klist for u in units]
                        else:
                            klist = ["DR"] + BF_TAPS
                            kloop = [(k, u) for u in units for k in klist]
                        for k, u in kloop:
                            rg, roff, nrows = u
                            pos = klist.index(k)
                            h0 = rg * ROWS_PER_TILE + roff
                            if k == "DR":
                                lhsT8 = w8r[:, :, cb * 128 : cb * 128 + 128]
                                win = x8_r[:, h0 : h0 + nrows, 0:64]
                                rhs8 = AP(
                                    win.tensor,
                                    win.offset,
                                    [win.ap[0], [2, 2], win.ap[1], win.ap[2]],
                                )
                                last_mm = nc.tensor.matmul(
                                    ps[u][:],
                                    lhsT8,
                                    rhs8,
                                    start=(pos == 0),
                                    stop=(pos == len(klist) - 1),
                                    perf_mode=DR,
                                )
                            else:
                                dh, dw = k // 3 - 1, k % 3 - 1
                                lhsT = weff[
                                    :, k * 256 + cb * 128 : k * 256 + cb * 128 + 128
                                ]
                                rhs = x_r[
                                    :,
                                    h0 + 1 + dh : h0 + 1 + dh + nrows,
                                    1 + dw : 65 + dw,
                                ]
                                last_mm = nc.tensor.matmul(
                                    ps[u][:],
                                    lhsT,
                                    rhs,
                                    start=(pos == 0),
                                    stop=(pos == len(klist) - 1),
                                )
                        # drain: PSUM -> bf16 SBUF (+bias) per unit,
                        # alternating ACT/DVE; one out-DMA per rg PAIR.
                        # The FINAL wave uses one DMA per unit on
                        # alternating queues so only 32KB drains after the
                        # final matmul.
                        if not last_wave:
                            prs = [
                                tuple(units[i : i + 2])
                                for i in range(0, len(units), 2)
                            ]
                        else:
                            prs = [(u,) for u in units]
                        for pi, pair in enumerate(prs):
                            wid = sum(64 * u[2] for u in pair)
                            o = outp.tile(
                                [128, wid], BF16, tag="o",
                                name=f"o{img}_{cb}_{pair[0][0]}_{pair[0][1]}",
                            )
                            col0 = pair[0][0] * 512 + pair[0][1] * 64
                            oc = 0
                            for h, u in enumerate(pair):
                                uw = 64 * u[2]
                                ti = (img * 2 + cb) * N_RG + u[0] + pi
                                if (ti + (h if not last_wave else 0)) % 2 == 0:
                                    nc.scalar.activation(
                                        o[:, oc : oc + uw],
                                        ps[u][:],
                                        IDENT,
                                        bias=b_sb[:, cb : cb + 1],
                                    )
                                else:
                                    nc.vector.tensor_scalar_add(
                                        o[:, oc : oc + uw],
                                        ps[u][:],
                                        b_sb[:, cb : cb + 1],
                                    )
                                oc += uw
                            dst = out[
                                img,
                                cb * 128 : (cb + 1) * 128,
                                col0 : col0 + wid,
                            ]
                            qs[pi % 2].dma_start(dst, o[:])
                        # remaining fp8 x pieces, derived on ACT between
                        # drains, each emitted one-plus waves before its
                        # first DoubleRow consumer.
                        if img == 0 and cb == 0 and wv == 0:
                            nc.scalar.copy(
                                x8_sb[0][:, 32 * WP : 48 * WP],
                                x_sb[0][:, 32 * WP : 48 * WP],
                            )
                        elif img == 0 and cb == 0 and wv == 1:
                            nc.scalar.copy(
                                x8_sb[0][:, 48 * WP :], x_sb[0][:, 48 * WP :]
                            )
                        elif img == 0 and cb == 1 and wv == 3:
                            nc.scalar.copy(x8_sb[1][:, :xa], x_sb[1][:, :xa])
                        elif img == 1 and cb == 0 and wv == 0:
                            nc.scalar.copy(x8_sb[1][:, xa:], x_sb[1][:, xa:])
                        first_wave = False

            # --- clock-hold fillers ---------------------------------------
            # Four f32 matmuls after the last conv matmul keep the HAM duty
            # cycle at 8/8 while the final drains + the semaphore parade run
            # (throttle hysteresis ~2.2us). They are data-independent, so
            # they MUST be chained behind the last conv matmul — the tile
            # scheduler otherwise hoists them to the head of the kernel.
            prev = last_mm
            for j in range(10):
                fps = psum.tile(
                    [128, 512], F32, tag="ps", bufs=5, name=f"fill{j}"
                )
                f = nc.tensor.matmul(
                    fps[:],
                    weff[:, 0:128],
                    x_sb[1][:, 0:512],
                    start=True,
                    stop=True,
                )
                add_dep_helper(f.ins, prev.ins, sync=True, reason="tail filler order")
                prev = f

    nc.compile()
    return nc


_NC_CACHE = None


def _get_nc():
    global _NC_CACHE
    if _NC_CACHE is None:
        _NC_CACHE = _build_nc()
    return _NC_CACHE


def _host_prep(x, W, b, lora_A, lora_B):
    """Layout + dtype rounding on host (RNE casts identical to what the
    on-device DVE/ACT converters produce); no other arithmetic."""
    x = np.ascontiguousarray(x, dtype=np.float32)
    xp_all = np.zeros((B, C_IN, HP, WP), dtype=np.float32)
    xp_all[:, :, 1 : H + 1, 1 : W_DIM + 1] = x
    xp_all = xp_all.reshape(B, C_IN, HP * WP).astype(BF16_NP)

    # [co, ci, kh, kw] -> [ci, k, co]
    wt = (
        np.ascontiguousarray(
            np.asarray(W, dtype=np.float32).reshape(C_OUT, C_IN, 9).transpose(1, 2, 0)
        )
        .reshape(C_IN, 9 * C_OUT)
        .astype(BF16_NP)
    )
    # lora_A [r, ci*9+k] -> [r, k, ci]; lora_B [co, r] -> [r, co]; bundled
    at = np.asarray(lora_A, dtype=np.float32).reshape(RANK, C_IN, 9).transpose(0, 2, 1)
    bt = np.asarray(lora_B, dtype=np.float32).T
    ab = np.concatenate(
        [at.reshape(RANK, 9 * C_IN), bt, np.zeros((RANK, C_OUT), np.float32)], axis=1
    ).astype(BF16_NP)
    ab = np.ascontiguousarray(ab)
    # [256] -> [128, 2]: bv[p, cb] = b[cb*128 + p]
    bv = np.ascontiguousarray(np.asarray(b, dtype=np.float32).reshape(2, 128).T)
    return xp_all, wt, ab, bv


def run(x, W, b, lora_A, lora_B, trace=False):
    """Run the kernel on 8 cores; returns (full_output, BassKernelResults)."""
    xp_all, wt, ab, bv = _host_prep(x, W, b, lora_A, lora_B)
    nc = _get_nc()
    in_maps = []
    for c in range(N_CORES):
        in_maps.append(
            {
                "xp": np.ascontiguousarray(xp_all[c * B_LOC : (c + 1) * B_LOC]),
                "wt": wt,
                "ab": ab,
                "bv": bv,
            }
        )
    res = run_bass_kernel_spmd(
        nc, in_maps, core_ids=list(range(N_CORES)), trace=trace
    )
    out = np.concatenate(
        [r["out"].astype(np.float32) for r in res.results], axis=0
    )
    return out.reshape(B, C_OUT, H, W_DIM), res


def kernel(x, W, b, lora_A, lora_B):
    out, _ = run(x, W, b, lora_A, lora_B, trace=False)
    return out
